# revision 1
# baseline (speedup 1.0000x reference)
"""Mistral-style GQA attention block (B=1, S=2048, HID=4096, 32 q heads /
8 kv heads, head_dim=128, RoPE, causal) on 8 Trainium2 NeuronCores.

Sharding: tensor-parallel over heads. Core c owns q heads [4c, 4c+4) and
kv head c: Wq/Wk/Wv column-sharded, Wo row-sharded; the o_proj partial
products are summed on the host (the all-reduce of the TP scheme).

Device kernel layout notes (per core):
  All matmuls use natural operand layouts -- no on-device transposes of
  activations except V (16 small PE transposes):
    Q^T[d,m] = Wq_chunk.T @ X^T_chunk      (d on partitions)
    S^T[k,q] = (K^T chunk).T @ Q^T chunk   (softmax runs over partitions)
    colsums  = ones.T @ exp(S^T)           (PE reduction over partitions)
    O^T[d,q] = V_chunk.T @ exp(S^T)
    Y[m,n]   = (O^T chunk).T @ Wo chunk
  Causality is exploited by only computing lower-triangle k-tiles; the
  four distinct diagonal-staircase mask patterns are sliced from the
  provided attention_mask input on the host.
  float32r (full fp32 data, reduced-precision PE mode, 4x faster than
  plain fp32) is used for the large matmuls.
"""

import os
import numpy as np
from contextlib import ExitStack

import concourse.bass as bass
from concourse import bacc
import concourse.tile as tile
from concourse import mybir
from concourse.bass_utils import run_bass_kernel_spmd
from concourse.masks import make_identity

AF = mybir.ActivationFunctionType
F32 = mybir.dt.float32
F32R = mybir.dt.float32r

S = 2048          # sequence length
HID = 4096        # hidden size
D = 128           # head dim
NCORES = 8
HPC = 4           # q heads per core
DPC = HPC * D     # 512 q-proj columns per core
MC = 512          # seq chunk (free dim of most matmuls)
NKC = HID // 128  # 32 contraction chunks for projections
NJC = S // MC     # 4 q chunks
NMT = S // 128    # 16 seq tiles of 128
SCALE = float(1.0 / np.sqrt(D))
ROPE_THETA = 10000.0

LAST_RESULTS = None  # BassKernelResults of the most recent run (for test.py)


def _rope(nc, pool, out, src_ps, cos, sin, tag):
    """out = src*cos + rotate_half(src)*sin, all [128, MC]; src in PSUM.

    A single ACT copy drains the PSUM bank (frees it for the next
    accumulation group after one op); the 5-op rope chain then runs on
    DVE off the SBUF scratch, overlapped with the next chunk's matmuls.
    """
    lo = slice(0, 64)
    hi = slice(64, 128)
    src = pool.tile([128, MC], F32, tag=f"rsc{tag}", bufs=2, name=f"ropesrc{tag}")
    nc.scalar.activation(src, src_ps, AF.Copy)
    tmp = pool.tile([128, MC], F32, tag="ropetmp", bufs=2, name="ropetmp")
    # sin is HALF-SWAPPED on the host (sin_sw[d] = sin[(d+64)%128]) so both
    # SBUF inputs of each mul share a base partition (walrus constraint).
    nc.vector.tensor_mul(out, src, cos)
    nc.vector.tensor_mul(tmp[lo, :], src[hi, :], sin[hi, :])
    nc.vector.tensor_mul(tmp[hi, :], src[lo, :], sin[lo, :])
    nc.vector.tensor_sub(out[lo, :], out[lo, :], tmp[lo, :])
    nc.vector.tensor_add(out[hi, :], out[hi, :], tmp[hi, :])


def _emit(nc, xkv, wq, wo, cosT, sinT, maskT, ones_in, y, rscr, tc):
    with ExitStack() as ctx:
        const = ctx.enter_context(tc.tile_pool(name="const", bufs=1))

        ident = const.tile([128, 128], F32, name="ident")
        make_identity(nc, ident)
        ones_col = const.tile([128, 1], F32R, name="ones_col")
        nc.sync.dma_start(ones_col, ones_in)

        cos_sb = const.tile([D, S], F32, name="cos_sb")
        nc.sync.dma_start(cos_sb, cosT)
        sin_sb = const.tile([D, S], F32, name="sin_sb")
        nc.sync.dma_start(sin_sb, sinT)
        # mask_sb[:, MC*t + b][a] = mask(q=b, k=128*t+a): the 4 staircase
        # patterns used on the k-tiles straddling the causal diagonal.
        mask_sb = const.tile([128, 4 * MC], F32, name="mask_sb")
        for t in range(4):
            nc.sync.dma_start(mask_sb[:, MC * t:MC * (t + 1)],
                              maskT[128 * t:128 * (t + 1), :])

        qt = [const.tile([D, S], F32R, name=f"qt{h}") for h in range(HPC)]
        kt = const.tile([D, S], F32R, name="kt")
        vsb = const.tile([128, S], F32R, name="vsb")  # vsb[:, 128i:+128] = V rows 128i..

        # ---------------- Phase A: projections + RoPE + V transpose -------
        with tc.tile_pool(name="pa", bufs=1) as pa, \
             tc.tile_pool(name="pap", bufs=1, space="PSUM") as pap:
            FB = MC + 2 * D          # 768: one fused chunk
            NG = int(os.environ.get("KERNEL_XKV_GROUP", "1"))
            XB = int(os.environ.get("KERNEL_XKV_BUFS", "8"))
            GW = NG * FB             # one packed DMA group (NG kc chunks)
            wq_t = []
            for g in range(4):
                w = pa.tile([128, 8 * MC], F32R, tag=f"wq{g}", name=f"wq_t{g}")
                nc.sync.dma_start(w, wq[:, 8 * MC * g:8 * MC * (g + 1)])
                wq_t.append(w)
            for mc in range(NJC):
                ms = slice(MC * mc, MC * (mc + 1))
                ps_q = [pap.tile([128, MC], F32, tag=f"q{h}", name=f"ps_q{h}_{mc}")
                        for h in range(HPC)]
                ps_k = pap.tile([128, MC], F32, tag="k", name=f"ps_k_{mc}")
                ps_v = pap.tile([128, MC], F32, tag="v", name=f"ps_v_{mc}")
                ngrp = NKC // NG
                for kcg in range(ngrp):
                    big = pa.tile([128, GW], F32R, tag="xkv", bufs=XB,
                                  name=f"xkv_{mc}_{kcg}")
                    nc.sync.dma_start(big, xkv[:, GW * (ngrp * mc + kcg):
                                               GW * (ngrp * mc + kcg + 1)])
                    for c2 in range(NG):
                        kc = NG * kcg + c2
                        base = FB * c2
                        xt_ = big[:, base:base + MC]
                        wk_ = big[:, base + MC:base + MC + D]
                        wv_ = big[:, base + MC + D:base + FB]
                        wqc = wq_t[kc // 8][:, MC * (kc % 8):MC * (kc % 8 + 1)]
                        st = kc == 0
                        sp = kc == NKC - 1
                        for h in range(HPC):
                            nc.tensor.matmul(ps_q[h], wqc[:, D * h:D * (h + 1)],
                                             xt_, start=st, stop=sp)
                        nc.tensor.matmul(ps_k, wk_, xt_, start=st, stop=sp)
                        nc.tensor.matmul(ps_v, wv_, xt_, start=st, stop=sp)
                for h in range(HPC):
                    _rope(nc, pa, qt[h][:, ms], ps_q[h], cos_sb[:, ms], sin_sb[:, ms], h)
                _rope(nc, pa, kt[:, ms], ps_k, cos_sb[:, ms], sin_sb[:, ms], 'k')
                vt_ = pa.tile([128, MC], F32, tag="vt", bufs=2, name=f"vt_{mc}")
                nc.scalar.activation(vt_, ps_v, AF.Copy)
                for b in range(4):
                    ps_t = pap.tile([128, 128], F32, tag="tr", name=f"ps_tr_{mc}_{b}")
                    nc.tensor.transpose(ps_t, vt_[:, 128 * b:128 * (b + 1)], ident)
                    i = 4 * mc + b
                    nc.vector.tensor_copy(vsb[:, 128 * i:128 * (i + 1)], ps_t)

        # ---------------- Phase B: attention --------------------------------
        phases = os.environ.get("KERNEL_PHASES", "ABC")
        if "B" not in phases:
            return
        obc = ctx.enter_context(tc.tile_pool(name="obc", bufs=1))
        ot = [obc.tile([D, S], F32R, name=f"ot{h}") for h in range(HPC)]
        with tc.tile_pool(name="pb", bufs=1) as pb, \
             tc.tile_pool(name="pbp", bufs=1, space="PSUM") as pbp:
            for h in range(HPC):
                for jc in range(NJC):
                    qs = slice(MC * jc, MC * (jc + 1))
                    nk = 4 * jc + 4
                    ps_o = pbp.tile([128, MC], F32, tag="o", bufs=int(os.environ.get("KERNEL_O_BUFS","2")), name=f"ps_o_{h}_{jc}")
                    ps_sum = pbp.tile([1, MC], F32, tag="sum", bufs=2, name=f"ps_sum_{h}_{jc}")
                    for i in range(nk):
                        ks = slice(128 * i, 128 * (i + 1))
                        ps_s = pbp.tile([128, MC], F32, tag="s", bufs=int(os.environ.get("KERNEL_S_BUFS","4")), name=f"ps_s_{h}_{jc}_{i}")
                        nc.tensor.matmul(ps_s, kt[:, ks], qt[h][:, qs],
                                         start=True, stop=True)
                        tt = i - 4 * jc
                        if tt >= 0:
                            nc.vector.tensor_add(ps_s, ps_s,
                                                 mask_sb[:, MC * tt:MC * (tt + 1)])
                        ex = pb.tile([128, MC], F32R, tag="ex", bufs=int(os.environ.get("KERNEL_EX_BUFS","6")), name=f"ex_{h}_{jc}_{i}")
                        nc.scalar.activation(ex, ps_s, AF.Exp, scale=SCALE)
                        st = i == 0
                        sp = i == nk - 1
                        nc.tensor.matmul(ps_o, vsb[:, ks], ex, start=st, stop=sp)
                        nc.tensor.matmul(ps_sum, ones_col, ex, start=st, stop=sp)
                    recip = pb.tile([1, MC], F32, tag="recip", bufs=2, name=f"recip_{h}_{jc}")
                    nc.vector.reciprocal(recip, ps_sum)
                    # broadcast recip over partitions via a DRAM bounce (off PE)
                    scr = rscr[4 * h + jc]
                    nc.sync.dma_start(scr, recip)
                    bcast = pb.tile([128, MC], F32, tag="bcast", bufs=2, name=f"bcast_{h}_{jc}")
                    nc.sync.dma_start(bcast, scr.to_broadcast((128, MC)))
                    nc.vector.tensor_mul(ot[h][:, qs], ps_o, bcast)

        # ---------------- Phase C: o_proj (row-sharded partial) -------------
        if "C" not in phases:
            return
        with tc.tile_pool(name="pc", bufs=1) as pc, \
             tc.tile_pool(name="pcp", bufs=1, space="PSUM") as pcp:
            HH = HID // 2
            for half in range(2):
                wo_t = [[None] * 4 for _ in range(HPC)]
                for dc in range(HPC):
                    for nq in range(4):
                        w = pc.tile([128, 512], F32R, tag=f"wo{dc}_{nq}",
                                    name=f"wo_{half}_{dc}_{nq}")
                        nc.sync.dma_start(
                            w, wo[128 * dc:128 * (dc + 1),
                                  HH * half + 512 * nq:HH * half + 512 * (nq + 1)])
                        wo_t[dc][nq] = w
                for mt in range(NMT):
                    yrow = pc.tile([128, HH], F32, tag="yrow", bufs=int(os.environ.get("KERNEL_YROW_BUFS","4")),
                                   name=f"yrow_{half}_{mt}")
                    for nq in range(4):
                        ps_y = pcp.tile([128, 512], F32, tag="y", bufs=int(os.environ.get("KERNEL_Y_BUFS","8")),
                                        name=f"ps_y_{half}_{mt}_{nq}")
                        for dc in range(HPC):
                            nc.tensor.matmul(ps_y, ot[dc][:, 128 * mt:128 * (mt + 1)],
                                             wo_t[dc][nq], start=(dc == 0),
                                             stop=(dc == HPC - 1))
                        nc.scalar.activation(yrow[:, 512 * nq:512 * (nq + 1)],
                                             ps_y, AF.Copy)
                    nc.sync.dma_start(y[128 * mt:128 * (mt + 1),
                                        HH * half:HH * (half + 1)], yrow)


_BUILT = None


def _build():
    global _BUILT
    if _BUILT is not None:
        return _BUILT
    nc = bacc.Bacc("TRN2", target_bir_lowering=False, debug=False,
                   num_devices=NCORES)
    xkv = nc.dram_tensor("xkv", [128, NJC * NKC * (MC + 2 * D)], F32R,
                         kind="ExternalInput").ap()
    wq = nc.dram_tensor("wq", [128, NKC * MC], F32R, kind="ExternalInput").ap()
    wo = nc.dram_tensor("wo", [DPC, HID], F32R, kind="ExternalInput").ap()
    cosT = nc.dram_tensor("cosT", [D, S], F32, kind="ExternalInput").ap()
    sinT = nc.dram_tensor("sinT", [D, S], F32, kind="ExternalInput").ap()
    maskT = nc.dram_tensor("maskT", [MC, MC], F32, kind="ExternalInput").ap()
    ones_in = nc.dram_tensor("ones_in", [128, 1], F32R, kind="ExternalInput").ap()
    y = nc.dram_tensor("y", [S, HID], F32, kind="ExternalOutput").ap()
    rscr = [nc.dram_tensor(f"rscr{i}", [1, MC], F32).ap() for i in range(16)]
    with tile.TileContext(nc) as tc:
        _emit(nc, xkv, wq, wo, cosT, sinT, maskT, ones_in, y, rscr, tc)
    nc.compile()
    _BUILT = nc
    return nc


def prep_in_maps(hidden_states, Wq, Wk, Wv, Wo, attention_mask, position_ids):
    hidden_states = np.asarray(hidden_states, dtype=np.float32)
    Wq = np.asarray(Wq, dtype=np.float32)
    Wk = np.asarray(Wk, dtype=np.float32)
    Wv = np.asarray(Wv, dtype=np.float32)
    Wo = np.asarray(Wo, dtype=np.float32)
    attention_mask = np.asarray(attention_mask, dtype=np.float32)
    position_ids = np.asarray(position_ids)

    xT = np.ascontiguousarray(hidden_states[0].T)  # [HID, S]

    # RoPE tables (host-precomputed from position_ids, as in the reference)
    pos = position_ids[0].astype(np.float32)  # [S]
    inv_freq = (1.0 / (ROPE_THETA ** (np.arange(0, D, 2, dtype=np.float32) / D))
                ).astype(np.float32)
    freqs = pos[:, None] * inv_freq[None, :]           # [S, D/2]
    emb = np.concatenate([freqs, freqs], axis=-1)      # [S, D]
    cosT = np.ascontiguousarray(np.cos(emb).T.astype(np.float32))  # [D, S]
    sinT = np.sin(emb).T.astype(np.float32)
    sinT = np.ascontiguousarray(np.concatenate([sinT[64:], sinT[:64]], axis=0))

    # diagonal staircase mask patterns, sliced from the provided mask
    maskT = np.ascontiguousarray(attention_mask[0, 0, :MC, :MC].T)  # [k, q]

    xTr = xT.reshape(NKC, 128, S)
    in_maps = []
    for c in range(NCORES):
        wk_c = Wk[:, D * c:D * (c + 1)].reshape(NKC, 128, D)
        wv_c = Wv[:, D * c:D * (c + 1)].reshape(NKC, 128, D)
        # blocks[mc, kc, p, j]: fused chunk = [xT cols | Wk | Wv]
        blocks = np.empty((NJC, NKC, 128, MC + 2 * D), dtype=np.float32)
        for mc in range(NJC):
            blocks[mc, :, :, :MC] = xTr[:, :, MC * mc:MC * (mc + 1)]
            blocks[mc, :, :, MC:MC + D] = wk_c
            blocks[mc, :, :, MC + D:] = wv_c
        # -> [p, mc, kc, j] flattened to the packed DMA layout
        xkv = blocks.transpose(2, 0, 1, 3).reshape(128, -1)
        wq_c = (Wq[:, DPC * c:DPC * (c + 1)].reshape(NKC, 128, DPC)
                .transpose(1, 0, 2).reshape(128, -1))
        in_maps.append({
            "xkv": np.ascontiguousarray(xkv),
            "wq": np.ascontiguousarray(wq_c),
            "wo": np.ascontiguousarray(Wo[DPC * c:DPC * (c + 1), :]),
            "cosT": cosT,
            "sinT": sinT,
            "maskT": maskT,
            "ones_in": np.ones((128, 1), dtype=np.float32),
        })

    return in_maps


def kernel(hidden_states, Wq, Wk, Wv, Wo, attention_mask, position_ids):
    global LAST_RESULTS
    in_maps = prep_in_maps(hidden_states, Wq, Wk, Wv, Wo, attention_mask,
                           position_ids)
    nc = _build()
    res = run_bass_kernel_spmd(nc, in_maps, list(range(NCORES)),
                               trace=bool(int(os.environ.get("KERNEL_TRACE", "0"))))
    LAST_RESULTS = res

    acc = np.zeros((S, HID), dtype=np.float64)
    for c in range(NCORES):
        acc += res.results[c]["y"].astype(np.float64)
    return acc.astype(np.float32)[None]  # [1, S, HID]

